# revision 34
# baseline (speedup 1.0000x reference)
"""3x3 MedianBlur (zero-padded) for (8, 3, 1024, 1024) fp32 on 8 trn2 NeuronCores.

v4: bf16 compute + band-pair "superband" instructions. The DVE runs
tensor_tensor at 2x for packed 2-byte dtypes (0.52 ns/elem vs 1.04 for fp32),
and bf16 keeps the median exact to ~2^-8 relative (selection network only --
no arithmetic), far inside the 2e-2 gate.

  - Pure data parallel: batch element i -> core i.
  - Per core: 8 row-bands of 128 rows; rows live in partitions, (channel, col)
    in the free dim. Vertical window alignment comes free from DMA: each band
    is loaded 3x from HBM at row offsets -1/0/+1 (fp32), then the ACT engine
    converts each to bf16. Pool does the zero-pad memsets; PE idle.
  - Exact separable median-of-9: 18 bf16 min/max tensor_tensor ops per band
    on the DVE (6-op vertical sort3 + 12-op horizontal merge), with in-place
    col-aligned tile reuse. This network is optimal for this machine: an
    exhaustive search proves med3(A,C,med3(m)) needs 8 gates, and MCMC
    searches over streaming networks (fixed V / fixed V+A+C / free) found
    nothing below 18.
  - Superbands: the middle six bands are processed as three band PAIRS living
    side by side in one tile [128, 2, 3, 1026]; each of the 18 ops covers both
    bands in ONE instruction, halving the per-instruction SBUF-access charge
    (58 cycles engine-busy each) and the semaphore traffic. Bands 0 and 7 stay
    single (chunked) for pipeline fill and drain. Compute scratch (m1/M1/lo/t2)
    is single-buffered: reuse hazards serialize on the in-order DVE for free.
  - Engine legality (measured): Pool cannot run TensorTensor (walrus codegen
    "engine check failed" -> NEFF fails to load), ACT bias/scale are
    per-partition scalars only, PE is linear-only. So all 18 ops sit on the
    DVE. Cost-model makespan 247.9us = fill 2.8us + DVE busy 242.0us (zero
    gaps) + drain 3.0us; the ends are at their structural floors (DMA start
    latency ~1.6-1.9us, DMA sem prop 900ns, barriers).
  - Steady-state queue plan: input loads on SP, input bf16 conversions on
    ACT, output f32 conversions on Pool (tensor_copy), output stores on SP.
    Keeping ACT free of store work matters: input conversions otherwise
    queue behind out-conversions that wait on compute, and the load->convert
    chain falls behind the 2-band compute period and starves the DVE.
  - Bands 1 and 2 run as half passes (band 1 computes while band 2 loads);
    a joint pass would stall ~10us in the fill on band 2's conversions,
    which sit behind the single-buffered f32 staging chain.
  - Tail drain: the last band splits at col 928; its channel stores fan out
    over the ACT, SP and Pool DMA queues (ch2's f32 conversion runs on the
    DVE at the 2x_2p TensorCopy rate), and the final 96-col piece writes f32
    directly into its own tile (outfB) so earlier stores see no false dep on
    the last DVE op. Output conversions live in a bufs=3 pool so band t's
    conversion never waits on band t-1's store DMA completing.
  - Band 0's first piece covers cols 0..518 (sweep-tuned, fill ~3.4us).

The walrus build accepts at most 1 inline sync wait per instruction
(2 on EventSemaphore); Tile emits more, so _legalize_waits() spills excess
waits onto same-engine NoOps placed immediately before the instruction.
"""
import sys

sys.path.insert(0, "/opt/trn_rl_repo")

import numpy as np

import concourse.bass as bass
import concourse.mybir as mybir
from concourse.bass_utils import run_bass_kernel_spmd
from concourse.tile import TileContext


C, H, W = 3, 1024, 1024
P = 128
NT = H // P
SP = W + 2      # padded width: tile col c <-> DRAM col c-1
F32 = mybir.dt.float32
BF16 = mybir.dt.bfloat16
MIN = mybir.AluOpType.min
MAX = mybir.AluOpType.max


def _legalize_waits(nc):
    """Split sync_info.on_wait lists that exceed this walrus's per-instruction
    capacity (1; 2 for EventSemaphore) onto preceding same-engine NoOps."""
    for f in nc.m.functions:
        for bb in f.blocks:
            new_insts = []
            for ins in bb.instructions:
                si = ins.sync_info
                cap = 2 if ins.opcode == "EventSemaphore" else 1
                if si is not None and len(si.on_wait) > cap:
                    waits = list(si.on_wait)
                    extra, keep = waits[:-cap], waits[-cap:]
                    for w in extra:
                        nop = mybir.InstNoOp(
                            name=nc.get_next_instruction_name(),
                            ins=[],
                            outs=[],
                            engine=ins.engine,
                        )
                        nop.sync_info = mybir.SyncInfo(on_wait=[w], on_update=[])
                        new_insts.append(nop)
                    ins.sync_info = mybir.SyncInfo(
                        on_wait=keep, on_update=list(si.on_update)
                    )
                new_insts.append(ins)
            bb.instructions = new_insts


def _hv(tile, h, c0, c1, a, b):
    """3D view [P, c1-c0, b-a] of band-half h of a super tile [P, 2, C, SP]."""
    e = tile[:, h : h + 1, c0:c1, a:b].copy()
    ap = [tuple(e.ap[0])] + [tuple(d) for d in e.ap[2:]]
    e.ap = e.ap.__class__(ap)
    return e


def build(bufs=2, s0=520):
    nc = bass.Bass()
    x = nc.dram_tensor("x", [C, H, W], F32, kind="ExternalInput")
    y = nc.dram_tensor("y", [C, H, W], F32, kind="ExternalOutput")
    tt = nc.vector.tensor_tensor

    with TileContext(nc) as tc:
        with (
            tc.tile_pool(name="pool", bufs=bufs) as pool,
            tc.tile_pool(name="spool", bufs=1) as spool,
            tc.tile_pool(name="fpool", bufs=1) as fpool,
            tc.tile_pool(name="opool", bufs=3) as opool,
        ):
            sup = {}  # S -> dict of super tiles [P, 2, C, SP]

            def alloc_super(S):
                xm = pool.tile([P, 2, C, SP], BF16, tag="xm")
                x0 = pool.tile([P, 2, C, SP], BF16, tag="x0")
                xp = pool.tile([P, 2, C, SP], BF16, tag="xp")
                sup[S] = {"xm": xm, "x0": x0, "xp": xp}
                # zero pad: tile cols 0 and 1025 of every (band, chan)
                for z in (xm, x0, xp):
                    e = z[:].copy()
                    e.ap = e.ap.__class__(
                        [tuple(e.ap[0]), (C * SP, 2), (SP, C), (W + 1, 2)]
                    )
                    nc.gpsimd.memset(e, 0.0)
                return sup[S]

            def emit_load(t, S, h, chunks):
                """Loads (f32) + bf16 conversion + pad memsets for band t into
                half h of super S. chunks: (c0, c1, w0, w1) pieces."""
                r0 = t * P
                d = sup[S] if S in sup else alloc_super(S)
                xm, x0, xp = d["xm"], d["x0"], d["xp"]
                xmf = fpool.tile([P, C, SP], F32, tag="xmf")
                x0f = fpool.tile([P, C, SP], F32, tag="x0f")
                xpf = fpool.tile([P, C, SP], F32, tag="xpf")
                for c0, c1, w0, w1 in chunks:
                    ta, tb = w0 + 1, w1 + 1  # tile col range of this piece
                    # pad rows: memset the f32 staging pad row before
                    # conversion (compute APs must start at partition 0, so
                    # [0:1] is legal but [1:P] is not; conversions cover [0:P])
                    if t == 0:
                        nc.gpsimd.memset(xmf[0:1, c0:c1, ta:tb], 0.0)
                    if t == NT - 1:
                        # base must be 32-aligned; DMA rewrites rows 96..126
                        nc.gpsimd.memset(xpf[96:P, c0:c1, ta:tb], 0.0)
                    # xm first: V's first two ops need only xm+x0, so their
                    # conversions gate DVE start. xm: rows r0-1..r0+126
                    if t == 0:
                        nc.sync.dma_start(
                            xmf[1:P, c0:c1, ta:tb],
                            x[c0:c1, 0 : P - 1, w0:w1].rearrange("c r w -> r c w"),
                        )
                    else:
                        nc.sync.dma_start(
                            xmf[:, c0:c1, ta:tb],
                            x[c0:c1, r0 - 1 : r0 + P - 1, w0:w1].rearrange(
                                "c r w -> r c w"
                            ),
                        )
                    # x0: rows r0..r0+127 (sync queue: keep the ACT SEQ
                    # free of HWDGE-serialized dma_starts so cvts run early)
                    nc.sync.dma_start(
                        x0f[:, c0:c1, ta:tb],
                        x[c0:c1, r0 : r0 + P, w0:w1].rearrange("c r w -> r c w"),
                    )
                    # xp: rows r0+1..r0+128
                    if t == NT - 1:
                        nc.sync.dma_start(
                            xpf[0 : P - 1, c0:c1, ta:tb],
                            x[c0:c1, r0 + 1 : r0 + P, w0:w1].rearrange(
                                "c r w -> r c w"
                            ),
                        )
                    else:
                        nc.sync.dma_start(
                            xpf[:, c0:c1, ta:tb],
                            x[c0:c1, r0 + 1 : r0 + P + 1, w0:w1].rearrange(
                                "c r w -> r c w"
                            ),
                        )
                    # bf16 conversions on ACT (full partition range; pad cols
                    # handled by the bf16 memsets above). Exception: the very
                    # first piece converts on the idle DVE -- it skips the
                    # ACT->DVE semaphore hop that gates the pipeline fill.
                    cvt = (
                        nc.vector.tensor_copy
                        if (t == 0 and (c0, c1, w0, w1) == chunks[0])
                        else nc.scalar.copy
                    )
                    cvt(_hv(xm, h, c0, c1, ta, tb), xmf[:, c0:c1, ta:tb])
                    cvt(_hv(x0, h, c0, c1, ta, tb), x0f[:, c0:c1, ta:tb])
                    cvt(_hv(xp, h, c0, c1, ta, tb), xpf[:, c0:c1, ta:tb])

            def emit_stores(t, lo, h, oc, outfB=None, fuse_last=False, dq="a"):
                r0 = t * P
                outf = opool.tile([P, C, W], F32, tag="outf")
                for ch in oc:
                    c0, c1, w0, w1 = ch[:4]
                    q = ch[4] if len(ch) > 4 else dq
                    ce = ch[5] if len(ch) > 5 else "a"
                    if not (fuse_last and ch == oc[-1]):
                        cvt = {
                            "a": nc.scalar.copy,
                            "v": nc.vector.tensor_copy,
                            "p": nc.gpsimd.tensor_copy,
                        }[ce]
                        cvt(outf[:, c0:c1, w0:w1], _hv(lo, h, c0, c1, w0, w1))
                    eng = {"a": nc.scalar, "s": nc.sync, "p": nc.gpsimd}[q]
                    if fuse_last and ch == oc[-1]:
                        src = outfB[:, c0:c1, 0 : w1 - w0]
                    else:
                        src = outf[:, c0:c1, w0:w1]
                    eng.dma_start(
                        y[c0:c1, r0 : r0 + P, w0:w1].rearrange("c r w -> r c w"),
                        src,
                    )

            def alloc_scratch():
                m1 = spool.tile([P, 2, C, SP], BF16, tag="m1")
                M1 = spool.tile([P, 2, C, SP], BF16, tag="M1")
                # lo is read by the (async) store-conversion path after the
                # band's compute, so it needs 2 slots or the next super's
                # first lo-write serializes behind that conversion
                lo = pool.tile([P, 2, C, SP], BF16, tag="lo")
                t2 = spool.tile([P, 2, C, SP], BF16, tag="t2")
                return m1, M1, lo, t2

            def net(v, o, c0, c1, w0, w1, outfB=None, last_piece=False):
                """The 18-op network on views: v = dict of slicer functions."""
                xm, x0, xp, m1, M1, lo, t2 = (
                    v["xm"], v["x0"], v["xp"], v["m1"], v["M1"], v["lo"], v["t2"]
                )
                va, vb = w0, w1 + 2
                tt(m1(c0, c1, va, vb), xm(c0, c1, va, vb), x0(c0, c1, va, vb), MIN)
                tt(M1(c0, c1, va, vb), xm(c0, c1, va, vb), x0(c0, c1, va, vb), MAX)
                tt(lo(c0, c1, va, vb), m1(c0, c1, va, vb), xp(c0, c1, va, vb), MIN)
                tt(t2(c0, c1, va, vb), M1(c0, c1, va, vb), xp(c0, c1, va, vb), MIN)
                tt(m1(c0, c1, va, vb), m1(c0, c1, va, vb), t2(c0, c1, va, vb), MAX)  # me
                tt(M1(c0, c1, va, vb), M1(c0, c1, va, vb), xp(c0, c1, va, vb), MAX)  # hi
                me, hi = m1, M1
                # stage H over out cols [w0, w1); reuse dead tiles:
                # pa->xm, pc->xp, q->x0, p->t2 (names = storage). Partials are
                # confined to [w0, w1): the w1'th value is never used, and
                # writing it would clobber the aliased x tiles at the next
                # col-chunk's V halo.
                h1 = w1 + 1
                tt(xm(c0, c1, w0, w1), lo(c0, c1, w0, w1), lo(c0, c1, w0 + 1, h1), MAX)  # pa
                tt(xp(c0, c1, w0, w1), hi(c0, c1, w0, w1), hi(c0, c1, w0 + 1, h1), MIN)  # pc
                tt(x0(c0, c1, w0, w1), me(c0, c1, w0, w1), me(c0, c1, w0 + 1, h1), MAX)  # q
                tt(t2(c0, c1, w0, w1), me(c0, c1, w0, w1), me(c0, c1, w0 + 1, h1), MIN)  # p
                tt(xm(c0, c1, w0, w1), xm(c0, c1, w0, w1), lo(c0, c1, w0 + 2, vb), MAX)  # A
                tt(xp(c0, c1, w0, w1), xp(c0, c1, w0, w1), hi(c0, c1, w0 + 2, vb), MIN)  # Cc
                tt(x0(c0, c1, w0, w1), x0(c0, c1, w0, w1), me(c0, c1, w0 + 2, vb), MIN)  # b1
                tt(x0(c0, c1, w0, w1), t2(c0, c1, w0, w1), x0(c0, c1, w0, w1), MAX)  # B
                A, B, Cc = xm, x0, xp
                tt(lo(c0, c1, w0, w1), A(c0, c1, w0, w1), B(c0, c1, w0, w1), MIN)  # m2
                tt(xm(c0, c1, w0, w1), A(c0, c1, w0, w1), B(c0, c1, w0, w1), MAX)  # M2
                tt(xp(c0, c1, w0, w1), xm(c0, c1, w0, w1), Cc(c0, c1, w0, w1), MIN)  # t3
                if last_piece:
                    # final piece: write f32 directly (fp32 rate on this one
                    # op) into its own tile so earlier stores don't pick up a
                    # false dep on the last DVE op
                    tt(outfB[:, c0:c1, 0 : w1 - w0], xp(c0, c1, w0, w1), lo(c0, c1, w0, w1), MAX)
                else:
                    tt(lo(c0, c1, w0, w1), xp(c0, c1, w0, w1), lo(c0, c1, w0, w1), MAX)  # out

            def emit_compute_super(S, t0):
                """Both bands of super S (bands t0, t0+1) in one 18-op pass."""
                d = sup[S]
                m1, M1, lo, t2 = alloc_scratch()
                tiles = {"xm": d["xm"], "x0": d["x0"], "xp": d["xp"],
                         "m1": m1, "M1": M1, "lo": lo, "t2": t2}
                v = {k: (lambda z: lambda c0, c1, a, b: z[:, :, c0:c1, a:b])(z)
                     for k, z in tiles.items()}
                net(v, None, 0, C, 0, W)
                # steady-state stores ride the SP queue and their f32
                # conversions run on the (otherwise idle) Pool engine: the ACT
                # queue must stay clear for the next bands' input conversions,
                # or the load->convert chain falls behind the 2-band compute
                # period and starves the DVE between supers
                for h, t in ((0, t0), (1, t0 + 1)):
                    emit_stores(
                        t, lo, h,
                        ((0, 2, 0, W, "s", "p"), (2, 3, 0, W, "s", "p")),
                    )

            def emit_compute_half(S, h, t, chunks, store_chunks=None,
                                  fuse_last=False):
                d = sup[S]
                m1, M1, lo, t2 = alloc_scratch()
                tiles = {"xm": d["xm"], "x0": d["x0"], "xp": d["xp"],
                         "m1": m1, "M1": M1, "lo": lo, "t2": t2}
                v = {k: (lambda z: lambda c0, c1, a, b: _hv(z, h, c0, c1, a, b))(z)
                     for k, z in tiles.items()}
                if fuse_last:
                    outfB = opool.tile([P, C, W - TB], F32, tag="outfB", name="outfB")
                else:
                    outfB = None
                for ck in chunks:
                    c0, c1, w0, w1 = ck
                    net(v, None, c0, c1, w0, w1, outfB,
                        last_piece=(fuse_last and ck == chunks[-1]))
                if store_chunks is not None:
                    oc = store_chunks
                elif len(chunks) == 1:
                    oc = ((0, 2, 0, W), (2, 3, 0, W))
                else:
                    oc = chunks
                emit_stores(t, lo, h, oc, outfB, fuse_last)

            full = [(0, C, 0, W)]
            # band 0: small first piece so DVE starts early (load split 2 cols
            # past the compute split so piece 1's V halo stays in load 1)
            load0 = [(0, 1, 0, s0), (0, 1, s0, W), (1, 3, 0, W)]
            comp0 = [(0, 1, 0, s0 - 2), (0, 1, s0 - 2, W), (1, 3, 0, W)]
            # last band: small last piece for a short drain; stores fan out
            # across the ACT, SP and Pool queues so the drains run in parallel
            TB = 928  # tail split col
            tailc = [(0, 3, 0, TB), (0, 3, TB, W)]
            tail_store = [
                (0, 1, 0, TB, "a", "a"),
                (1, 2, 0, TB, "s", "a"),
                (2, 3, 0, TB, "p", "v"),
                (0, 3, TB, W, "a", "a"),
            ]
            # supers: S0 = band 0 (half), S1=(1,2), S2=(3,4), S3=(5,6),
            # S4 = band 7 (half)
            sstore = ((0, 2, 0, W, "s", "p"), (2, 3, 0, W, "s", "p"))
            emit_load(0, 0, 0, load0)
            emit_load(1, 1, 0, full)
            emit_compute_half(0, 0, 0, comp0)
            emit_load(2, 1, 1, full)
            emit_load(3, 2, 0, full)
            # bands 1 and 2 as half passes: band 1's compute can start while
            # band 2 is still loading (a joint super pass would stall ~10us
            # in the fill waiting for band 2's conversions)
            emit_compute_half(1, 0, 1, full, sstore)
            emit_compute_half(1, 1, 2, full, sstore)
            emit_load(4, 2, 1, full)
            emit_load(5, 3, 0, full)
            emit_compute_super(2, 3)
            emit_load(6, 3, 1, full)
            emit_load(7, 4, 0, full)
            emit_compute_super(3, 5)
            emit_compute_half(4, 0, 7, tailc, tail_store, fuse_last=True)

    _legalize_waits(nc)
    return nc


_NC = None


def kernel(input):
    global _NC
    if _NC is None:
        _NC = build()
    input = np.asarray(input, dtype=np.float32)
    in_maps = [{"x": np.ascontiguousarray(input[i])} for i in range(input.shape[0])]
    res = run_bass_kernel_spmd(_NC, in_maps, core_ids=list(range(len(in_maps))))
    return np.stack([r["y"] for r in res.results], axis=0)


# revision 39
# speedup vs baseline: 1.0013x; 1.0013x over previous
"""3x3 MedianBlur (zero-padded) for (8, 3, 1024, 1024) fp32 on 8 trn2 NeuronCores.

v4: bf16 compute + band-pair "superband" instructions. The DVE runs
tensor_tensor at 2x for packed 2-byte dtypes (0.52 ns/elem vs 1.04 for fp32),
and bf16 keeps the median exact to ~2^-8 relative (selection network only --
no arithmetic), far inside the 2e-2 gate.

  - Pure data parallel: batch element i -> core i.
  - Per core: 8 row-bands of 128 rows; rows live in partitions, (channel, col)
    in the free dim. Vertical window alignment comes free from DMA: each band
    is loaded 3x from HBM at row offsets -1/0/+1 (fp32), then the ACT engine
    converts each to bf16. Pool does the zero-pad memsets; PE idle.
  - Exact separable median-of-9: 18 bf16 min/max tensor_tensor ops per band
    on the DVE (6-op vertical sort3 + 12-op horizontal merge), with in-place
    col-aligned tile reuse. This network is optimal for this machine: an
    exhaustive search proves med3(A,C,med3(m)) needs 8 gates, and MCMC
    searches over streaming networks (fixed V / fixed V+A+C / free) found
    nothing below 18.
  - Superbands: the middle six bands are processed as three band PAIRS living
    side by side in one tile [128, 2, 3, 1026]; each of the 18 ops covers both
    bands in ONE instruction, halving the per-instruction SBUF-access charge
    (58 cycles engine-busy each) and the semaphore traffic. Bands 0 and 7 stay
    single (chunked) for pipeline fill and drain. Compute scratch (m1/M1/lo/t2)
    is single-buffered: reuse hazards serialize on the in-order DVE for free.
  - Engine legality (measured): Pool cannot run TensorTensor (walrus codegen
    "engine check failed" -> NEFF fails to load), ACT bias/scale are
    per-partition scalars only, PE is linear-only. So all 18 ops sit on the
    DVE. Cost-model makespan 247.9us = fill 2.8us + DVE busy 242.0us (zero
    gaps) + drain 3.0us; the ends are at their structural floors (DMA start
    latency ~1.6-1.9us, DMA sem prop 900ns, barriers).
  - Steady-state queue plan: input loads on SP, input bf16 conversions on
    ACT, output f32 conversions on Pool (tensor_copy), output stores on SP.
    Keeping ACT free of store work matters: input conversions otherwise
    queue behind out-conversions that wait on compute, and the load->convert
    chain falls behind the 2-band compute period and starves the DVE.
  - Bands 1 and 2 run as half passes (band 1 computes while band 2 loads);
    a joint pass would stall ~10us in the fill on band 2's conversions,
    which sit behind the single-buffered f32 staging chain.
  - Tail drain: the last band splits at col 928; its channel stores fan out
    over the ACT, SP and Pool DMA queues (ch2's f32 conversion runs on the
    DVE at the 2x_2p TensorCopy rate), and the final 96-col piece writes f32
    directly into its own tile (outfB) so earlier stores see no false dep on
    the last DVE op. Output conversions live in a bufs=3 pool so band t's
    conversion never waits on band t-1's store DMA completing.
  - Band 0's first piece covers cols 0..518 (sweep-tuned, fill ~3.4us).

The walrus build accepts at most 1 inline sync wait per instruction
(2 on EventSemaphore); Tile emits more, so _legalize_waits() spills excess
waits onto same-engine NoOps placed immediately before the instruction.
"""
import sys

sys.path.insert(0, "/opt/trn_rl_repo")

import numpy as np

import concourse.bass as bass
import concourse.mybir as mybir
from concourse.bass_utils import run_bass_kernel_spmd
from concourse.tile import TileContext


C, H, W = 3, 1024, 1024
P = 128
NT = H // P
SP = W + 2      # padded width: tile col c <-> DRAM col c-1
F32 = mybir.dt.float32
BF16 = mybir.dt.bfloat16
MIN = mybir.AluOpType.min
MAX = mybir.AluOpType.max


def _legalize_waits(nc):
    """Split sync_info.on_wait lists that exceed this walrus's per-instruction
    capacity (1; 2 for EventSemaphore) onto preceding same-engine NoOps."""
    for f in nc.m.functions:
        for bb in f.blocks:
            new_insts = []
            for ins in bb.instructions:
                si = ins.sync_info
                cap = 2 if ins.opcode == "EventSemaphore" else 1
                if si is not None and len(si.on_wait) > cap:
                    waits = list(si.on_wait)
                    extra, keep = waits[:-cap], waits[-cap:]
                    for w in extra:
                        nop = mybir.InstNoOp(
                            name=nc.get_next_instruction_name(),
                            ins=[],
                            outs=[],
                            engine=ins.engine,
                        )
                        nop.sync_info = mybir.SyncInfo(on_wait=[w], on_update=[])
                        new_insts.append(nop)
                    ins.sync_info = mybir.SyncInfo(
                        on_wait=keep, on_update=list(si.on_update)
                    )
                new_insts.append(ins)
            bb.instructions = new_insts


def _hv(tile, h, c0, c1, a, b):
    """3D view [P, c1-c0, b-a] of band-half h of a super tile [P, 2, C, SP]."""
    e = tile[:, h : h + 1, c0:c1, a:b].copy()
    ap = [tuple(e.ap[0])] + [tuple(d) for d in e.ap[2:]]
    e.ap = e.ap.__class__(ap)
    return e


def build(bufs=2, s0=520):
    nc = bass.Bass()
    x = nc.dram_tensor("x", [C, H, W], F32, kind="ExternalInput")
    y = nc.dram_tensor("y", [C, H, W], F32, kind="ExternalOutput")
    tt = nc.vector.tensor_tensor

    with TileContext(nc) as tc:
        with (
            tc.tile_pool(name="pool", bufs=bufs) as pool,
            tc.tile_pool(name="spool", bufs=1) as spool,
            tc.tile_pool(name="fpool", bufs=1) as fpool,
            tc.tile_pool(name="opool", bufs=3) as opool,
        ):
            sup = {}  # S -> dict of super tiles [P, 2, C, SP]

            def alloc_super(S):
                xm = pool.tile([P, 2, C, SP], BF16, tag="xm")
                x0 = pool.tile([P, 2, C, SP], BF16, tag="x0")
                xp = pool.tile([P, 2, C, SP], BF16, tag="xp")
                sup[S] = {"xm": xm, "x0": x0, "xp": xp}
                # zero pad: tile cols 0 and 1025 of every (band, chan)
                for z in (xm, x0, xp):
                    e = z[:].copy()
                    e.ap = e.ap.__class__(
                        [tuple(e.ap[0]), (C * SP, 2), (SP, C), (W + 1, 2)]
                    )
                    nc.gpsimd.memset(e, 0.0)
                return sup[S]

            def emit_load(t, S, h, chunks, lq=None, cq=None):
                """Loads (f32) + bf16 conversion + pad memsets for band t into
                half h of super S. chunks: (c0, c1, w0, w1) pieces.
                lq/cq: optional load-queue engine and conversion fn overrides
                (band 2 rides the Pool queue end-to-end so its data arrives in
                parallel with band 1's on SP/ACT during the pipeline fill)."""
                r0 = t * P
                d = sup[S] if S in sup else alloc_super(S)
                xm, x0, xp = d["xm"], d["x0"], d["xp"]
                if lq is None:
                    xmf = fpool.tile([P, C, SP], F32, tag="xmf")
                    x0f = fpool.tile([P, C, SP], F32, tag="x0f")
                    xpf = fpool.tile([P, C, SP], F32, tag="xpf")
                else:
                    # off-chain band: stage in the (idle-during-fill) outf
                    # slots so these loads skip the fpool reuse chain. Full
                    # chunks touch exactly 1024 staging cols, so the outf
                    # slot shape fits with a -1 column offset.
                    xmf = opool.tile([P, C, W], F32, tag="outf", name="xmf2")
                    x0f = opool.tile([P, C, W], F32, tag="outf", name="x0f2")
                    xpf = opool.tile([P, C, W], F32, tag="outf", name="xpf2")
                ld = lq if lq is not None else nc.sync
                so = 0 if lq is None else 1
                for c0, c1, w0, w1 in chunks:
                    ta, tb = w0 + 1, w1 + 1  # tile col range of this piece
                    # pad rows: memset the f32 staging pad row before
                    # conversion (compute APs must start at partition 0, so
                    # [0:1] is legal but [1:P] is not; conversions cover [0:P])
                    if t == 0:
                        nc.gpsimd.memset(xmf[0:1, c0:c1, ta - so : tb - so], 0.0)
                    if t == NT - 1:
                        # base must be 32-aligned; DMA rewrites rows 96..126
                        nc.gpsimd.memset(xpf[96:P, c0:c1, ta - so : tb - so], 0.0)
                    # xm first: V's first two ops need only xm+x0, so their
                    # conversions gate DVE start. xm: rows r0-1..r0+126
                    if t == 0:
                        ld.dma_start(
                            xmf[1:P, c0:c1, ta - so : tb - so],
                            x[c0:c1, 0 : P - 1, w0:w1].rearrange("c r w -> r c w"),
                        )
                    else:
                        ld.dma_start(
                            xmf[:, c0:c1, ta - so : tb - so],
                            x[c0:c1, r0 - 1 : r0 + P - 1, w0:w1].rearrange(
                                "c r w -> r c w"
                            ),
                        )
                    # x0: rows r0..r0+127 (sync queue: keep the ACT SEQ
                    # free of HWDGE-serialized dma_starts so cvts run early)
                    ld.dma_start(
                        x0f[:, c0:c1, ta - so : tb - so],
                        x[c0:c1, r0 : r0 + P, w0:w1].rearrange("c r w -> r c w"),
                    )
                    # xp: rows r0+1..r0+128
                    if t == NT - 1:
                        ld.dma_start(
                            xpf[0 : P - 1, c0:c1, ta - so : tb - so],
                            x[c0:c1, r0 + 1 : r0 + P, w0:w1].rearrange(
                                "c r w -> r c w"
                            ),
                        )
                    else:
                        ld.dma_start(
                            xpf[:, c0:c1, ta - so : tb - so],
                            x[c0:c1, r0 + 1 : r0 + P + 1, w0:w1].rearrange(
                                "c r w -> r c w"
                            ),
                        )
                    # bf16 conversions on ACT (full partition range; pad cols
                    # handled by the bf16 memsets above). Exception: the very
                    # first piece converts on the idle DVE -- it skips the
                    # ACT->DVE semaphore hop that gates the pipeline fill.
                    if cq is not None:
                        cvt = cq
                    elif t == 0 and (c0, c1, w0, w1) == chunks[0]:
                        cvt = nc.vector.tensor_copy
                    else:
                        cvt = nc.scalar.copy
                    cvt(_hv(xm, h, c0, c1, ta, tb), xmf[:, c0:c1, ta - so : tb - so])
                    cvt(_hv(x0, h, c0, c1, ta, tb), x0f[:, c0:c1, ta - so : tb - so])
                    cvt(_hv(xp, h, c0, c1, ta, tb), xpf[:, c0:c1, ta - so : tb - so])

            def emit_stores(t, lo, h, oc, outfB=None, fuse_last=False, dq="a"):
                r0 = t * P
                outf = opool.tile([P, C, W], F32, tag="outf")
                for ch in oc:
                    c0, c1, w0, w1 = ch[:4]
                    q = ch[4] if len(ch) > 4 else dq
                    ce = ch[5] if len(ch) > 5 else "a"
                    if not (fuse_last and ch == oc[-1]):
                        cvt = {
                            "a": nc.scalar.copy,
                            "v": nc.vector.tensor_copy,
                            "p": nc.gpsimd.tensor_copy,
                        }[ce]
                        cvt(outf[:, c0:c1, w0:w1], _hv(lo, h, c0, c1, w0, w1))
                    eng = {"a": nc.scalar, "s": nc.sync, "p": nc.gpsimd}[q]
                    if fuse_last and ch == oc[-1]:
                        src = outfB[:, c0:c1, 0 : w1 - w0]
                    else:
                        src = outf[:, c0:c1, w0:w1]
                    eng.dma_start(
                        y[c0:c1, r0 : r0 + P, w0:w1].rearrange("c r w -> r c w"),
                        src,
                    )

            def alloc_scratch():
                m1 = spool.tile([P, 2, C, SP], BF16, tag="m1")
                M1 = spool.tile([P, 2, C, SP], BF16, tag="M1")
                # lo is read by the (async) store-conversion path after the
                # band's compute, so it needs 2 slots or the next super's
                # first lo-write serializes behind that conversion
                lo = pool.tile([P, 2, C, SP], BF16, tag="lo")
                t2 = spool.tile([P, 2, C, SP], BF16, tag="t2")
                return m1, M1, lo, t2

            def net(v, o, c0, c1, w0, w1, outfB=None, last_piece=False):
                """The 18-op network on views: v = dict of slicer functions."""
                xm, x0, xp, m1, M1, lo, t2 = (
                    v["xm"], v["x0"], v["xp"], v["m1"], v["M1"], v["lo"], v["t2"]
                )
                va, vb = w0, w1 + 2
                tt(m1(c0, c1, va, vb), xm(c0, c1, va, vb), x0(c0, c1, va, vb), MIN)
                tt(M1(c0, c1, va, vb), xm(c0, c1, va, vb), x0(c0, c1, va, vb), MAX)
                tt(lo(c0, c1, va, vb), m1(c0, c1, va, vb), xp(c0, c1, va, vb), MIN)
                tt(t2(c0, c1, va, vb), M1(c0, c1, va, vb), xp(c0, c1, va, vb), MIN)
                tt(m1(c0, c1, va, vb), m1(c0, c1, va, vb), t2(c0, c1, va, vb), MAX)  # me
                tt(M1(c0, c1, va, vb), M1(c0, c1, va, vb), xp(c0, c1, va, vb), MAX)  # hi
                me, hi = m1, M1
                # stage H over out cols [w0, w1); reuse dead tiles:
                # pa->xm, pc->xp, q->x0, p->t2 (names = storage). Partials are
                # confined to [w0, w1): the w1'th value is never used, and
                # writing it would clobber the aliased x tiles at the next
                # col-chunk's V halo.
                h1 = w1 + 1
                tt(xm(c0, c1, w0, w1), lo(c0, c1, w0, w1), lo(c0, c1, w0 + 1, h1), MAX)  # pa
                tt(xp(c0, c1, w0, w1), hi(c0, c1, w0, w1), hi(c0, c1, w0 + 1, h1), MIN)  # pc
                tt(x0(c0, c1, w0, w1), me(c0, c1, w0, w1), me(c0, c1, w0 + 1, h1), MAX)  # q
                tt(t2(c0, c1, w0, w1), me(c0, c1, w0, w1), me(c0, c1, w0 + 1, h1), MIN)  # p
                tt(xm(c0, c1, w0, w1), xm(c0, c1, w0, w1), lo(c0, c1, w0 + 2, vb), MAX)  # A
                tt(xp(c0, c1, w0, w1), xp(c0, c1, w0, w1), hi(c0, c1, w0 + 2, vb), MIN)  # Cc
                tt(x0(c0, c1, w0, w1), x0(c0, c1, w0, w1), me(c0, c1, w0 + 2, vb), MIN)  # b1
                tt(x0(c0, c1, w0, w1), t2(c0, c1, w0, w1), x0(c0, c1, w0, w1), MAX)  # B
                A, B, Cc = xm, x0, xp
                tt(lo(c0, c1, w0, w1), A(c0, c1, w0, w1), B(c0, c1, w0, w1), MIN)  # m2
                tt(xm(c0, c1, w0, w1), A(c0, c1, w0, w1), B(c0, c1, w0, w1), MAX)  # M2
                tt(xp(c0, c1, w0, w1), xm(c0, c1, w0, w1), Cc(c0, c1, w0, w1), MIN)  # t3
                if last_piece:
                    # final piece: write f32 directly (fp32 rate on this one
                    # op) into its own tile so earlier stores don't pick up a
                    # false dep on the last DVE op
                    tt(outfB[:, c0:c1, 0 : w1 - w0], xp(c0, c1, w0, w1), lo(c0, c1, w0, w1), MAX)
                else:
                    tt(lo(c0, c1, w0, w1), xp(c0, c1, w0, w1), lo(c0, c1, w0, w1), MAX)  # out

            def emit_compute_super(S, t0):
                """Both bands of super S (bands t0, t0+1) in one 18-op pass."""
                d = sup[S]
                m1, M1, lo, t2 = alloc_scratch()
                tiles = {"xm": d["xm"], "x0": d["x0"], "xp": d["xp"],
                         "m1": m1, "M1": M1, "lo": lo, "t2": t2}
                v = {k: (lambda z: lambda c0, c1, a, b: z[:, :, c0:c1, a:b])(z)
                     for k, z in tiles.items()}
                net(v, None, 0, C, 0, W)
                # steady-state stores ride the SP queue and their f32
                # conversions run on the (otherwise idle) Pool engine: the ACT
                # queue must stay clear for the next bands' input conversions,
                # or the load->convert chain falls behind the 2-band compute
                # period and starves the DVE between supers
                for h, t in ((0, t0), (1, t0 + 1)):
                    emit_stores(
                        t, lo, h,
                        ((0, 2, 0, W, "s", "p"), (2, 3, 0, W, "s", "p")),
                    )

            def emit_compute_half(S, h, t, chunks, store_chunks=None,
                                  fuse_last=False):
                d = sup[S]
                m1, M1, lo, t2 = alloc_scratch()
                tiles = {"xm": d["xm"], "x0": d["x0"], "xp": d["xp"],
                         "m1": m1, "M1": M1, "lo": lo, "t2": t2}
                v = {k: (lambda z: lambda c0, c1, a, b: _hv(z, h, c0, c1, a, b))(z)
                     for k, z in tiles.items()}
                if fuse_last:
                    outfB = opool.tile([P, C, W - TB], F32, tag="outfB", name="outfB")
                else:
                    outfB = None
                for ck in chunks:
                    c0, c1, w0, w1 = ck
                    net(v, None, c0, c1, w0, w1, outfB,
                        last_piece=(fuse_last and ck == chunks[-1]))
                if store_chunks is not None:
                    oc = store_chunks
                elif len(chunks) == 1:
                    oc = ((0, 2, 0, W), (2, 3, 0, W))
                else:
                    oc = chunks
                emit_stores(t, lo, h, oc, outfB, fuse_last)

            full = [(0, C, 0, W)]
            # band 0: small first piece so DVE starts early (load split 2 cols
            # past the compute split so piece 1's V halo stays in load 1)
            load0 = [(0, 1, 0, s0), (0, 1, s0, W), (1, 3, 0, W)]
            comp0 = [(0, 1, 0, s0 - 2), (0, 1, s0 - 2, W), (1, 3, 0, W)]
            # last band: small last piece for a short drain; stores fan out
            # across the ACT, SP and Pool queues so the drains run in parallel
            TB = 928  # tail split col
            tailc = [(0, 3, 0, TB), (0, 3, TB, W)]
            tail_store = [
                (0, 1, 0, TB, "a", "a"),
                (1, 2, 0, TB, "s", "a"),
                (2, 3, 0, TB, "p", "v"),
                (0, 3, TB, W, "a", "a"),
            ]
            # supers: S0 = band 0 (half), S1=(1,2), S2=(3,4), S3=(5,6),
            # S4 = band 7 (half)
            sstore = ((0, 2, 0, W, "s", "p"), (2, 3, 0, W, "s", "p"))
            emit_load(0, 0, 0, load0)
            # band 2 rides the Pool queue end-to-end (loads, staging in the
            # idle outf slots, conversions) so it lands in parallel with
            # band 1's SP/ACT chain -- that makes the (1,2) super pass start
            # on time instead of stalling ~10us on band 2's conversions
            emit_load(2, 1, 1, full, lq=nc.gpsimd, cq=nc.gpsimd.tensor_copy)
            emit_load(1, 1, 0, full)
            emit_compute_half(0, 0, 0, comp0)
            emit_load(3, 2, 0, full)
            emit_compute_super(1, 1)
            emit_load(4, 2, 1, full)
            emit_load(5, 3, 0, full)
            emit_compute_super(2, 3)
            emit_load(6, 3, 1, full)
            emit_load(7, 4, 0, full)
            emit_compute_super(3, 5)
            emit_compute_half(4, 0, 7, tailc, tail_store, fuse_last=True)

    _legalize_waits(nc)
    return nc


_NC = None


def kernel(input):
    global _NC
    if _NC is None:
        _NC = build()
    input = np.asarray(input, dtype=np.float32)
    in_maps = [{"x": np.ascontiguousarray(input[i])} for i in range(input.shape[0])]
    res = run_bass_kernel_spmd(_NC, in_maps, core_ids=list(range(len(in_maps))))
    return np.stack([r["y"] for r in res.results], axis=0)


# revision 42
# speedup vs baseline: 1.0039x; 1.0026x over previous
"""3x3 MedianBlur (zero-padded) for (8, 3, 1024, 1024) fp32 on 8 trn2 NeuronCores.

v4: bf16 compute + band-pair "superband" instructions. The DVE runs
tensor_tensor at 2x for packed 2-byte dtypes (0.52 ns/elem vs 1.04 for fp32),
and bf16 keeps the median exact to ~2^-8 relative (selection network only --
no arithmetic), far inside the 2e-2 gate.

  - Pure data parallel: batch element i -> core i.
  - Per core: 8 row-bands of 128 rows; rows live in partitions, (channel, col)
    in the free dim. Vertical window alignment comes free from DMA: each band
    is loaded 3x from HBM at row offsets -1/0/+1 (fp32), then the ACT engine
    converts each to bf16. Pool does the zero-pad memsets; PE idle.
  - Exact separable median-of-9: 18 bf16 min/max tensor_tensor ops per band
    on the DVE (6-op vertical sort3 + 12-op horizontal merge), with in-place
    col-aligned tile reuse. This network is optimal for this machine: an
    exhaustive search proves med3(A,C,med3(m)) needs 8 gates, and MCMC
    searches over streaming networks (fixed V / fixed V+A+C / free) found
    nothing below 18.
  - Superbands: the middle six bands are processed as three band PAIRS living
    side by side in one tile [128, 2, 3, 1026]; each of the 18 ops covers both
    bands in ONE instruction, halving the per-instruction SBUF-access charge
    (58 cycles engine-busy each) and the semaphore traffic. Bands 0 and 7 stay
    single (chunked) for pipeline fill and drain. Compute scratch m1/M1/t2 is
    single-buffered (reuse hazards serialize on the in-order DVE for free);
    lo is double-buffered because the async store-conversion path reads it.
  - Engine legality (measured): Pool cannot run TensorTensor (walrus codegen
    "engine check failed" -> NEFF fails to load), ACT bias/scale are
    per-partition scalars only, PE is linear-only. So all 18 ops sit on the
    DVE. Cost-model makespan 247.5us = fill 2.8us + DVE busy 240.9us + drain
    3.0us; the ends are at their structural floors (DMA start latency
    ~1.6-1.9us, DMA sem prop 900ns, barriers).
  - Steady-state queue plan: input loads on SP, input bf16 conversions on
    ACT, output f32 conversions on Pool (tensor_copy), output stores on SP.
    Keeping ACT free of store work matters: input conversions otherwise
    queue behind out-conversions that wait on compute, and the load->convert
    chain falls behind the 2-band compute period and starves the DVE.
  - Fill: band 2 rides the Pool queue end-to-end (loads via SWDGE, staging
    in the idle outf slots with a -1 column offset, f32->bf16 conversions on
    Pool tensor_copy) so it lands in parallel with band 1's SP/ACT chain and
    the (1,2) super pass starts on time.
  - Tail drain: the last band splits at col 928; its channel stores fan out
    over the ACT, SP and Pool DMA queues (ch2's f32 conversion runs on the
    DVE at the 2x_2p TensorCopy rate), and the final 96-col piece writes f32
    directly into its own tile (outfB) so earlier stores see no false dep on
    the last DVE op. Output conversions live in a bufs=3 pool so band t's
    conversion never waits on band t-1's store DMA completing.
  - Band 0's first piece covers cols 0..558 (sweep-tuned); band 0's xp
    loads ride the ACT queue so piece 3's xm/x0 clear SP sooner (xp is only
    needed 3 ops into each piece's V stage). Makespan 246.9us.

The walrus build accepts at most 1 inline sync wait per instruction
(2 on EventSemaphore); Tile emits more, so _legalize_waits() spills excess
waits onto same-engine NoOps placed immediately before the instruction.
"""
import sys

sys.path.insert(0, "/opt/trn_rl_repo")

import numpy as np

import concourse.bass as bass
import concourse.mybir as mybir
from concourse.bass_utils import run_bass_kernel_spmd
from concourse.tile import TileContext


C, H, W = 3, 1024, 1024
P = 128
NT = H // P
SP = W + 2      # padded width: tile col c <-> DRAM col c-1
F32 = mybir.dt.float32
BF16 = mybir.dt.bfloat16
MIN = mybir.AluOpType.min
MAX = mybir.AluOpType.max


def _legalize_waits(nc):
    """Split sync_info.on_wait lists that exceed this walrus's per-instruction
    capacity (1; 2 for EventSemaphore) onto preceding same-engine NoOps."""
    for f in nc.m.functions:
        for bb in f.blocks:
            new_insts = []
            for ins in bb.instructions:
                si = ins.sync_info
                cap = 2 if ins.opcode == "EventSemaphore" else 1
                if si is not None and len(si.on_wait) > cap:
                    waits = list(si.on_wait)
                    extra, keep = waits[:-cap], waits[-cap:]
                    for w in extra:
                        nop = mybir.InstNoOp(
                            name=nc.get_next_instruction_name(),
                            ins=[],
                            outs=[],
                            engine=ins.engine,
                        )
                        nop.sync_info = mybir.SyncInfo(on_wait=[w], on_update=[])
                        new_insts.append(nop)
                    ins.sync_info = mybir.SyncInfo(
                        on_wait=keep, on_update=list(si.on_update)
                    )
                new_insts.append(ins)
            bb.instructions = new_insts


def _hv(tile, h, c0, c1, a, b):
    """3D view [P, c1-c0, b-a] of band-half h of a super tile [P, 2, C, SP]."""
    e = tile[:, h : h + 1, c0:c1, a:b].copy()
    ap = [tuple(e.ap[0])] + [tuple(d) for d in e.ap[2:]]
    e.ap = e.ap.__class__(ap)
    return e


def build(bufs=2, s0=560):
    nc = bass.Bass()
    x = nc.dram_tensor("x", [C, H, W], F32, kind="ExternalInput")
    y = nc.dram_tensor("y", [C, H, W], F32, kind="ExternalOutput")
    tt = nc.vector.tensor_tensor

    with TileContext(nc) as tc:
        with (
            tc.tile_pool(name="pool", bufs=bufs) as pool,
            tc.tile_pool(name="spool", bufs=1) as spool,
            tc.tile_pool(name="fpool", bufs=1) as fpool,
            tc.tile_pool(name="opool", bufs=3) as opool,
        ):
            sup = {}  # S -> dict of super tiles [P, 2, C, SP]

            def alloc_super(S):
                xm = pool.tile([P, 2, C, SP], BF16, tag="xm")
                x0 = pool.tile([P, 2, C, SP], BF16, tag="x0")
                xp = pool.tile([P, 2, C, SP], BF16, tag="xp")
                sup[S] = {"xm": xm, "x0": x0, "xp": xp}
                # zero pad: tile cols 0 and 1025 of every (band, chan)
                for z in (xm, x0, xp):
                    e = z[:].copy()
                    e.ap = e.ap.__class__(
                        [tuple(e.ap[0]), (C * SP, 2), (SP, C), (W + 1, 2)]
                    )
                    nc.gpsimd.memset(e, 0.0)
                return sup[S]

            def emit_load(t, S, h, chunks, lq=None, cq=None, xpq=None):
                """Loads (f32) + bf16 conversion + pad memsets for band t into
                half h of super S. chunks: (c0, c1, w0, w1) pieces.
                lq/cq: optional load-queue engine and conversion fn overrides
                (band 2 rides the Pool queue end-to-end so its data arrives in
                parallel with band 1's on SP/ACT during the pipeline fill)."""
                r0 = t * P
                d = sup[S] if S in sup else alloc_super(S)
                xm, x0, xp = d["xm"], d["x0"], d["xp"]
                if lq is None:
                    xmf = fpool.tile([P, C, SP], F32, tag="xmf")
                    x0f = fpool.tile([P, C, SP], F32, tag="x0f")
                    xpf = fpool.tile([P, C, SP], F32, tag="xpf")
                else:
                    # off-chain band: stage in the (idle-during-fill) outf
                    # slots so these loads skip the fpool reuse chain. Full
                    # chunks touch exactly 1024 staging cols, so the outf
                    # slot shape fits with a -1 column offset.
                    xmf = opool.tile([P, C, W], F32, tag="outf", name="xmf2")
                    x0f = opool.tile([P, C, W], F32, tag="outf", name="x0f2")
                    xpf = opool.tile([P, C, W], F32, tag="outf", name="xpf2")
                ld = lq if lq is not None else nc.sync
                ldp = xpq if xpq is not None else ld
                so = 0 if lq is None else 1
                for c0, c1, w0, w1 in chunks:
                    ta, tb = w0 + 1, w1 + 1  # tile col range of this piece
                    # pad rows: memset the f32 staging pad row before
                    # conversion (compute APs must start at partition 0, so
                    # [0:1] is legal but [1:P] is not; conversions cover [0:P])
                    if t == 0:
                        nc.gpsimd.memset(xmf[0:1, c0:c1, ta - so : tb - so], 0.0)
                    if t == NT - 1:
                        # base must be 32-aligned; DMA rewrites rows 96..126
                        nc.gpsimd.memset(xpf[96:P, c0:c1, ta - so : tb - so], 0.0)
                    # xm first: V's first two ops need only xm+x0, so their
                    # conversions gate DVE start. xm: rows r0-1..r0+126
                    if t == 0:
                        ld.dma_start(
                            xmf[1:P, c0:c1, ta - so : tb - so],
                            x[c0:c1, 0 : P - 1, w0:w1].rearrange("c r w -> r c w"),
                        )
                    else:
                        ld.dma_start(
                            xmf[:, c0:c1, ta - so : tb - so],
                            x[c0:c1, r0 - 1 : r0 + P - 1, w0:w1].rearrange(
                                "c r w -> r c w"
                            ),
                        )
                    # x0: rows r0..r0+127 (sync queue: keep the ACT SEQ
                    # free of HWDGE-serialized dma_starts so cvts run early)
                    ld.dma_start(
                        x0f[:, c0:c1, ta - so : tb - so],
                        x[c0:c1, r0 : r0 + P, w0:w1].rearrange("c r w -> r c w"),
                    )
                    # xp: rows r0+1..r0+128
                    if t == NT - 1:
                        ldp.dma_start(
                            xpf[0 : P - 1, c0:c1, ta - so : tb - so],
                            x[c0:c1, r0 + 1 : r0 + P, w0:w1].rearrange(
                                "c r w -> r c w"
                            ),
                        )
                    else:
                        ldp.dma_start(
                            xpf[:, c0:c1, ta - so : tb - so],
                            x[c0:c1, r0 + 1 : r0 + P + 1, w0:w1].rearrange(
                                "c r w -> r c w"
                            ),
                        )
                    # bf16 conversions on ACT (full partition range; pad cols
                    # handled by the bf16 memsets above). Exception: the very
                    # first piece converts on the idle DVE -- it skips the
                    # ACT->DVE semaphore hop that gates the pipeline fill.
                    if cq is not None:
                        cvt = cq
                    elif t == 0 and (c0, c1, w0, w1) == chunks[0]:
                        cvt = nc.vector.tensor_copy
                    else:
                        cvt = nc.scalar.copy
                    cvt(_hv(xm, h, c0, c1, ta, tb), xmf[:, c0:c1, ta - so : tb - so])
                    cvt(_hv(x0, h, c0, c1, ta, tb), x0f[:, c0:c1, ta - so : tb - so])
                    cvt(_hv(xp, h, c0, c1, ta, tb), xpf[:, c0:c1, ta - so : tb - so])

            def emit_stores(t, lo, h, oc, outfB=None, fuse_last=False, dq="a"):
                r0 = t * P
                outf = opool.tile([P, C, W], F32, tag="outf")
                for ch in oc:
                    c0, c1, w0, w1 = ch[:4]
                    q = ch[4] if len(ch) > 4 else dq
                    ce = ch[5] if len(ch) > 5 else "a"
                    if not (fuse_last and ch == oc[-1]):
                        cvt = {
                            "a": nc.scalar.copy,
                            "v": nc.vector.tensor_copy,
                            "p": nc.gpsimd.tensor_copy,
                        }[ce]
                        cvt(outf[:, c0:c1, w0:w1], _hv(lo, h, c0, c1, w0, w1))
                    eng = {"a": nc.scalar, "s": nc.sync, "p": nc.gpsimd}[q]
                    if fuse_last and ch == oc[-1]:
                        src = outfB[:, c0:c1, 0 : w1 - w0]
                    else:
                        src = outf[:, c0:c1, w0:w1]
                    eng.dma_start(
                        y[c0:c1, r0 : r0 + P, w0:w1].rearrange("c r w -> r c w"),
                        src,
                    )

            def alloc_scratch():
                m1 = spool.tile([P, 2, C, SP], BF16, tag="m1")
                M1 = spool.tile([P, 2, C, SP], BF16, tag="M1")
                # lo is read by the (async) store-conversion path after the
                # band's compute, so it needs 2 slots or the next super's
                # first lo-write serializes behind that conversion
                lo = pool.tile([P, 2, C, SP], BF16, tag="lo")
                t2 = spool.tile([P, 2, C, SP], BF16, tag="t2")
                return m1, M1, lo, t2

            def net(v, o, c0, c1, w0, w1, outfB=None, last_piece=False):
                """The 18-op network on views: v = dict of slicer functions."""
                xm, x0, xp, m1, M1, lo, t2 = (
                    v["xm"], v["x0"], v["xp"], v["m1"], v["M1"], v["lo"], v["t2"]
                )
                va, vb = w0, w1 + 2
                tt(m1(c0, c1, va, vb), xm(c0, c1, va, vb), x0(c0, c1, va, vb), MIN)
                tt(M1(c0, c1, va, vb), xm(c0, c1, va, vb), x0(c0, c1, va, vb), MAX)
                tt(lo(c0, c1, va, vb), m1(c0, c1, va, vb), xp(c0, c1, va, vb), MIN)
                tt(t2(c0, c1, va, vb), M1(c0, c1, va, vb), xp(c0, c1, va, vb), MIN)
                tt(m1(c0, c1, va, vb), m1(c0, c1, va, vb), t2(c0, c1, va, vb), MAX)  # me
                tt(M1(c0, c1, va, vb), M1(c0, c1, va, vb), xp(c0, c1, va, vb), MAX)  # hi
                me, hi = m1, M1
                # stage H over out cols [w0, w1); reuse dead tiles:
                # pa->xm, pc->xp, q->x0, p->t2 (names = storage). Partials are
                # confined to [w0, w1): the w1'th value is never used, and
                # writing it would clobber the aliased x tiles at the next
                # col-chunk's V halo.
                h1 = w1 + 1
                tt(xm(c0, c1, w0, w1), lo(c0, c1, w0, w1), lo(c0, c1, w0 + 1, h1), MAX)  # pa
                tt(xp(c0, c1, w0, w1), hi(c0, c1, w0, w1), hi(c0, c1, w0 + 1, h1), MIN)  # pc
                tt(x0(c0, c1, w0, w1), me(c0, c1, w0, w1), me(c0, c1, w0 + 1, h1), MAX)  # q
                tt(t2(c0, c1, w0, w1), me(c0, c1, w0, w1), me(c0, c1, w0 + 1, h1), MIN)  # p
                tt(xm(c0, c1, w0, w1), xm(c0, c1, w0, w1), lo(c0, c1, w0 + 2, vb), MAX)  # A
                tt(xp(c0, c1, w0, w1), xp(c0, c1, w0, w1), hi(c0, c1, w0 + 2, vb), MIN)  # Cc
                tt(x0(c0, c1, w0, w1), x0(c0, c1, w0, w1), me(c0, c1, w0 + 2, vb), MIN)  # b1
                tt(x0(c0, c1, w0, w1), t2(c0, c1, w0, w1), x0(c0, c1, w0, w1), MAX)  # B
                A, B, Cc = xm, x0, xp
                tt(lo(c0, c1, w0, w1), A(c0, c1, w0, w1), B(c0, c1, w0, w1), MIN)  # m2
                tt(xm(c0, c1, w0, w1), A(c0, c1, w0, w1), B(c0, c1, w0, w1), MAX)  # M2
                tt(xp(c0, c1, w0, w1), xm(c0, c1, w0, w1), Cc(c0, c1, w0, w1), MIN)  # t3
                if last_piece:
                    # final piece: write f32 directly (fp32 rate on this one
                    # op) into its own tile so earlier stores don't pick up a
                    # false dep on the last DVE op
                    tt(outfB[:, c0:c1, 0 : w1 - w0], xp(c0, c1, w0, w1), lo(c0, c1, w0, w1), MAX)
                else:
                    tt(lo(c0, c1, w0, w1), xp(c0, c1, w0, w1), lo(c0, c1, w0, w1), MAX)  # out

            def emit_compute_super(S, t0):
                """Both bands of super S (bands t0, t0+1) in one 18-op pass."""
                d = sup[S]
                m1, M1, lo, t2 = alloc_scratch()
                tiles = {"xm": d["xm"], "x0": d["x0"], "xp": d["xp"],
                         "m1": m1, "M1": M1, "lo": lo, "t2": t2}
                v = {k: (lambda z: lambda c0, c1, a, b: z[:, :, c0:c1, a:b])(z)
                     for k, z in tiles.items()}
                net(v, None, 0, C, 0, W)
                # steady-state stores ride the SP queue and their f32
                # conversions run on the (otherwise idle) Pool engine: the ACT
                # queue must stay clear for the next bands' input conversions,
                # or the load->convert chain falls behind the 2-band compute
                # period and starves the DVE between supers
                for h, t in ((0, t0), (1, t0 + 1)):
                    emit_stores(
                        t, lo, h,
                        ((0, 2, 0, W, "s", "p"), (2, 3, 0, W, "s", "p")),
                    )

            def emit_compute_half(S, h, t, chunks, store_chunks=None,
                                  fuse_last=False):
                d = sup[S]
                m1, M1, lo, t2 = alloc_scratch()
                tiles = {"xm": d["xm"], "x0": d["x0"], "xp": d["xp"],
                         "m1": m1, "M1": M1, "lo": lo, "t2": t2}
                v = {k: (lambda z: lambda c0, c1, a, b: _hv(z, h, c0, c1, a, b))(z)
                     for k, z in tiles.items()}
                if fuse_last:
                    outfB = opool.tile([P, C, W - TB], F32, tag="outfB", name="outfB")
                else:
                    outfB = None
                for ck in chunks:
                    c0, c1, w0, w1 = ck
                    net(v, None, c0, c1, w0, w1, outfB,
                        last_piece=(fuse_last and ck == chunks[-1]))
                if store_chunks is not None:
                    oc = store_chunks
                elif len(chunks) == 1:
                    oc = ((0, 2, 0, W), (2, 3, 0, W))
                else:
                    oc = chunks
                emit_stores(t, lo, h, oc, outfB, fuse_last)

            full = [(0, C, 0, W)]
            # band 0: small first piece so DVE starts early (load split 2 cols
            # past the compute split so piece 1's V halo stays in load 1)
            load0 = [(0, 1, 0, s0), (0, 1, s0, W), (1, 3, 0, W)]
            comp0 = [(0, 1, 0, s0 - 2), (0, 1, s0 - 2, W), (1, 3, 0, W)]
            # last band: small last piece for a short drain; stores fan out
            # across the ACT, SP and Pool queues so the drains run in parallel
            TB = 928  # tail split col
            tailc = [(0, 3, 0, TB), (0, 3, TB, W)]
            tail_store = [
                (0, 1, 0, TB, "a", "a"),
                (1, 2, 0, TB, "s", "a"),
                (2, 3, 0, TB, "p", "v"),
                (0, 3, TB, W, "a", "a"),
            ]
            # supers: S0 = band 0 (half), S1=(1,2), S2=(3,4), S3=(5,6),
            # S4 = band 7 (half)
            sstore = ((0, 2, 0, W, "s", "p"), (2, 3, 0, W, "s", "p"))
            # band 0's xp loads ride the ACT queue: piece 3's xm/x0 then
            # clear the SP queue sooner, trimming the fill hiccup before its
            # compute (xp is only needed 3 ops into each piece's V stage)
            emit_load(0, 0, 0, load0, xpq=nc.scalar)
            # band 2 rides the Pool queue end-to-end (loads, staging in the
            # idle outf slots, conversions) so it lands in parallel with
            # band 1's SP/ACT chain -- that makes the (1,2) super pass start
            # on time instead of stalling ~10us on band 2's conversions
            emit_load(2, 1, 1, full, lq=nc.gpsimd, cq=nc.gpsimd.tensor_copy)
            emit_load(1, 1, 0, full)
            emit_compute_half(0, 0, 0, comp0)
            emit_load(3, 2, 0, full)
            emit_compute_super(1, 1)
            emit_load(4, 2, 1, full)
            emit_load(5, 3, 0, full)
            emit_compute_super(2, 3)
            emit_load(6, 3, 1, full)
            emit_load(7, 4, 0, full)
            emit_compute_super(3, 5)
            emit_compute_half(4, 0, 7, tailc, tail_store, fuse_last=True)

    _legalize_waits(nc)
    return nc


_NC = None


def kernel(input):
    global _NC
    if _NC is None:
        _NC = build()
    input = np.asarray(input, dtype=np.float32)
    in_maps = [{"x": np.ascontiguousarray(input[i])} for i in range(input.shape[0])]
    res = run_bass_kernel_spmd(_NC, in_maps, core_ids=list(range(len(in_maps))))
    return np.stack([r["y"] for r in res.results], axis=0)


# revision 43
# speedup vs baseline: 1.0049x; 1.0010x over previous
"""3x3 MedianBlur (zero-padded) for (8, 3, 1024, 1024) fp32 on 8 trn2 NeuronCores.

v4: bf16 compute + band-pair "superband" instructions. The DVE runs
tensor_tensor at 2x for packed 2-byte dtypes (0.52 ns/elem vs 1.04 for fp32),
and bf16 keeps the median exact to ~2^-8 relative (selection network only --
no arithmetic), far inside the 2e-2 gate.

  - Pure data parallel: batch element i -> core i.
  - Per core: 8 row-bands of 128 rows; rows live in partitions, (channel, col)
    in the free dim. Vertical window alignment comes free from DMA: each band
    is loaded 3x from HBM at row offsets -1/0/+1 (fp32), then the ACT engine
    converts each to bf16. Pool does the zero-pad memsets; PE idle.
  - Exact separable median-of-9: 18 bf16 min/max tensor_tensor ops per band
    on the DVE (6-op vertical sort3 + 12-op horizontal merge), with in-place
    col-aligned tile reuse. This network is optimal for this machine: an
    exhaustive search proves med3(A,C,med3(m)) needs 8 gates, and MCMC
    searches over streaming networks (fixed V / fixed V+A+C / free) found
    nothing below 18.
  - Superbands: the middle six bands are processed as three band PAIRS living
    side by side in one tile [128, 2, 3, 1026]; each of the 18 ops covers both
    bands in ONE instruction, halving the per-instruction SBUF-access charge
    (58 cycles engine-busy each) and the semaphore traffic. Bands 0 and 7 stay
    single (chunked) for pipeline fill and drain. Compute scratch m1/M1/t2 is
    single-buffered (reuse hazards serialize on the in-order DVE for free);
    lo is double-buffered because the async store-conversion path reads it.
  - Engine legality (measured): Pool cannot run TensorTensor (walrus codegen
    "engine check failed" -> NEFF fails to load), ACT bias/scale are
    per-partition scalars only, PE is linear-only. So all 18 ops sit on the
    DVE. Cost-model makespan 247.5us = fill 2.8us + DVE busy 240.9us + drain
    3.0us; the ends are at their structural floors (DMA start latency
    ~1.6-1.9us, DMA sem prop 900ns, barriers).
  - Steady-state queue plan: input loads on SP, input bf16 conversions on
    ACT, output f32 conversions on Pool (tensor_copy), output stores on SP.
    Keeping ACT free of store work matters: input conversions otherwise
    queue behind out-conversions that wait on compute, and the load->convert
    chain falls behind the 2-band compute period and starves the DVE.
  - Fill: band 2 rides the Pool queue end-to-end (loads via SWDGE, staging
    in the idle outf slots with a -1 column offset, f32->bf16 conversions on
    Pool tensor_copy) so it lands in parallel with band 1's SP/ACT chain and
    the (1,2) super pass starts on time.
  - Tail drain: the last band splits at col 928; its channel stores fan out
    over the ACT, SP and Pool DMA queues (ch2's f32 conversion runs on the
    DVE at the 2x_2p TensorCopy rate), and the final 96-col piece writes f32
    directly into its own tile (outfB) so earlier stores see no false dep on
    the last DVE op. Output conversions live in a bufs=3 pool so band t's
    conversion never waits on band t-1's store DMA completing.
  - Band 0's first piece covers cols 0..478 (sweep-tuned); band 0's xp
    loads ride the ACT queue so piece 3's xm/x0 clear SP sooner (xp is only
    needed 3 ops into each piece's V stage). Makespan 246.7us.

The walrus build accepts at most 1 inline sync wait per instruction
(2 on EventSemaphore); Tile emits more, so _legalize_waits() spills excess
waits onto same-engine NoOps placed immediately before the instruction.
"""
import sys

sys.path.insert(0, "/opt/trn_rl_repo")

import numpy as np

import concourse.bass as bass
import concourse.mybir as mybir
from concourse.bass_utils import run_bass_kernel_spmd
from concourse.tile import TileContext


C, H, W = 3, 1024, 1024
P = 128
NT = H // P
SP = W + 2      # padded width: tile col c <-> DRAM col c-1
F32 = mybir.dt.float32
BF16 = mybir.dt.bfloat16
MIN = mybir.AluOpType.min
MAX = mybir.AluOpType.max


def _legalize_waits(nc):
    """Split sync_info.on_wait lists that exceed this walrus's per-instruction
    capacity (1; 2 for EventSemaphore) onto preceding same-engine NoOps."""
    for f in nc.m.functions:
        for bb in f.blocks:
            new_insts = []
            for ins in bb.instructions:
                si = ins.sync_info
                cap = 2 if ins.opcode == "EventSemaphore" else 1
                if si is not None and len(si.on_wait) > cap:
                    waits = list(si.on_wait)
                    extra, keep = waits[:-cap], waits[-cap:]
                    for w in extra:
                        nop = mybir.InstNoOp(
                            name=nc.get_next_instruction_name(),
                            ins=[],
                            outs=[],
                            engine=ins.engine,
                        )
                        nop.sync_info = mybir.SyncInfo(on_wait=[w], on_update=[])
                        new_insts.append(nop)
                    ins.sync_info = mybir.SyncInfo(
                        on_wait=keep, on_update=list(si.on_update)
                    )
                new_insts.append(ins)
            bb.instructions = new_insts


def _hv(tile, h, c0, c1, a, b):
    """3D view [P, c1-c0, b-a] of band-half h of a super tile [P, 2, C, SP]."""
    e = tile[:, h : h + 1, c0:c1, a:b].copy()
    ap = [tuple(e.ap[0])] + [tuple(d) for d in e.ap[2:]]
    e.ap = e.ap.__class__(ap)
    return e


def build(bufs=2, s0=480):
    nc = bass.Bass()
    x = nc.dram_tensor("x", [C, H, W], F32, kind="ExternalInput")
    y = nc.dram_tensor("y", [C, H, W], F32, kind="ExternalOutput")
    tt = nc.vector.tensor_tensor

    with TileContext(nc) as tc:
        with (
            tc.tile_pool(name="pool", bufs=bufs) as pool,
            tc.tile_pool(name="spool", bufs=1) as spool,
            tc.tile_pool(name="fpool", bufs=1) as fpool,
            tc.tile_pool(name="opool", bufs=3) as opool,
        ):
            sup = {}  # S -> dict of super tiles [P, 2, C, SP]

            def alloc_super(S):
                xm = pool.tile([P, 2, C, SP], BF16, tag="xm")
                x0 = pool.tile([P, 2, C, SP], BF16, tag="x0")
                xp = pool.tile([P, 2, C, SP], BF16, tag="xp")
                sup[S] = {"xm": xm, "x0": x0, "xp": xp}
                # zero pad: tile cols 0 and 1025 of every (band, chan)
                for z in (xm, x0, xp):
                    e = z[:].copy()
                    e.ap = e.ap.__class__(
                        [tuple(e.ap[0]), (C * SP, 2), (SP, C), (W + 1, 2)]
                    )
                    nc.gpsimd.memset(e, 0.0)
                return sup[S]

            def emit_load(t, S, h, chunks, lq=None, cq=None, xpq=None):
                """Loads (f32) + bf16 conversion + pad memsets for band t into
                half h of super S. chunks: (c0, c1, w0, w1) pieces.
                lq/cq: optional load-queue engine and conversion fn overrides
                (band 2 rides the Pool queue end-to-end so its data arrives in
                parallel with band 1's on SP/ACT during the pipeline fill)."""
                r0 = t * P
                d = sup[S] if S in sup else alloc_super(S)
                xm, x0, xp = d["xm"], d["x0"], d["xp"]
                if lq is None:
                    xmf = fpool.tile([P, C, SP], F32, tag="xmf")
                    x0f = fpool.tile([P, C, SP], F32, tag="x0f")
                    xpf = fpool.tile([P, C, SP], F32, tag="xpf")
                else:
                    # off-chain band: stage in the (idle-during-fill) outf
                    # slots so these loads skip the fpool reuse chain. Full
                    # chunks touch exactly 1024 staging cols, so the outf
                    # slot shape fits with a -1 column offset.
                    xmf = opool.tile([P, C, W], F32, tag="outf", name="xmf2")
                    x0f = opool.tile([P, C, W], F32, tag="outf", name="x0f2")
                    xpf = opool.tile([P, C, W], F32, tag="outf", name="xpf2")
                ld = lq if lq is not None else nc.sync
                ldp = xpq if xpq is not None else ld
                so = 0 if lq is None else 1
                for c0, c1, w0, w1 in chunks:
                    ta, tb = w0 + 1, w1 + 1  # tile col range of this piece
                    # pad rows: memset the f32 staging pad row before
                    # conversion (compute APs must start at partition 0, so
                    # [0:1] is legal but [1:P] is not; conversions cover [0:P])
                    if t == 0:
                        nc.gpsimd.memset(xmf[0:1, c0:c1, ta - so : tb - so], 0.0)
                    if t == NT - 1:
                        # base must be 32-aligned; DMA rewrites rows 96..126
                        nc.gpsimd.memset(xpf[96:P, c0:c1, ta - so : tb - so], 0.0)
                    # xm first: V's first two ops need only xm+x0, so their
                    # conversions gate DVE start. xm: rows r0-1..r0+126
                    if t == 0:
                        ld.dma_start(
                            xmf[1:P, c0:c1, ta - so : tb - so],
                            x[c0:c1, 0 : P - 1, w0:w1].rearrange("c r w -> r c w"),
                        )
                    else:
                        ld.dma_start(
                            xmf[:, c0:c1, ta - so : tb - so],
                            x[c0:c1, r0 - 1 : r0 + P - 1, w0:w1].rearrange(
                                "c r w -> r c w"
                            ),
                        )
                    # x0: rows r0..r0+127 (sync queue: keep the ACT SEQ
                    # free of HWDGE-serialized dma_starts so cvts run early)
                    ld.dma_start(
                        x0f[:, c0:c1, ta - so : tb - so],
                        x[c0:c1, r0 : r0 + P, w0:w1].rearrange("c r w -> r c w"),
                    )
                    # xp: rows r0+1..r0+128
                    if t == NT - 1:
                        ldp.dma_start(
                            xpf[0 : P - 1, c0:c1, ta - so : tb - so],
                            x[c0:c1, r0 + 1 : r0 + P, w0:w1].rearrange(
                                "c r w -> r c w"
                            ),
                        )
                    else:
                        ldp.dma_start(
                            xpf[:, c0:c1, ta - so : tb - so],
                            x[c0:c1, r0 + 1 : r0 + P + 1, w0:w1].rearrange(
                                "c r w -> r c w"
                            ),
                        )
                    # bf16 conversions on ACT (full partition range; pad cols
                    # handled by the bf16 memsets above). Exception: the very
                    # first piece converts on the idle DVE -- it skips the
                    # ACT->DVE semaphore hop that gates the pipeline fill.
                    if cq is not None:
                        cvt = cq
                    elif t == 0 and (c0, c1, w0, w1) == chunks[0]:
                        cvt = nc.vector.tensor_copy
                    else:
                        cvt = nc.scalar.copy
                    cvt(_hv(xm, h, c0, c1, ta, tb), xmf[:, c0:c1, ta - so : tb - so])
                    cvt(_hv(x0, h, c0, c1, ta, tb), x0f[:, c0:c1, ta - so : tb - so])
                    cvt(_hv(xp, h, c0, c1, ta, tb), xpf[:, c0:c1, ta - so : tb - so])

            def emit_stores(t, lo, h, oc, outfB=None, fuse_last=False, dq="a"):
                r0 = t * P
                outf = opool.tile([P, C, W], F32, tag="outf")
                for ch in oc:
                    c0, c1, w0, w1 = ch[:4]
                    q = ch[4] if len(ch) > 4 else dq
                    ce = ch[5] if len(ch) > 5 else "a"
                    if not (fuse_last and ch == oc[-1]):
                        cvt = {
                            "a": nc.scalar.copy,
                            "v": nc.vector.tensor_copy,
                            "p": nc.gpsimd.tensor_copy,
                        }[ce]
                        cvt(outf[:, c0:c1, w0:w1], _hv(lo, h, c0, c1, w0, w1))
                    eng = {"a": nc.scalar, "s": nc.sync, "p": nc.gpsimd}[q]
                    if fuse_last and ch == oc[-1]:
                        src = outfB[:, c0:c1, 0 : w1 - w0]
                    else:
                        src = outf[:, c0:c1, w0:w1]
                    eng.dma_start(
                        y[c0:c1, r0 : r0 + P, w0:w1].rearrange("c r w -> r c w"),
                        src,
                    )

            def alloc_scratch():
                m1 = spool.tile([P, 2, C, SP], BF16, tag="m1")
                M1 = spool.tile([P, 2, C, SP], BF16, tag="M1")
                # lo is read by the (async) store-conversion path after the
                # band's compute, so it needs 2 slots or the next super's
                # first lo-write serializes behind that conversion
                lo = pool.tile([P, 2, C, SP], BF16, tag="lo")
                t2 = spool.tile([P, 2, C, SP], BF16, tag="t2")
                return m1, M1, lo, t2

            def net(v, o, c0, c1, w0, w1, outfB=None, last_piece=False):
                """The 18-op network on views: v = dict of slicer functions."""
                xm, x0, xp, m1, M1, lo, t2 = (
                    v["xm"], v["x0"], v["xp"], v["m1"], v["M1"], v["lo"], v["t2"]
                )
                va, vb = w0, w1 + 2
                tt(m1(c0, c1, va, vb), xm(c0, c1, va, vb), x0(c0, c1, va, vb), MIN)
                tt(M1(c0, c1, va, vb), xm(c0, c1, va, vb), x0(c0, c1, va, vb), MAX)
                tt(lo(c0, c1, va, vb), m1(c0, c1, va, vb), xp(c0, c1, va, vb), MIN)
                tt(t2(c0, c1, va, vb), M1(c0, c1, va, vb), xp(c0, c1, va, vb), MIN)
                tt(m1(c0, c1, va, vb), m1(c0, c1, va, vb), t2(c0, c1, va, vb), MAX)  # me
                tt(M1(c0, c1, va, vb), M1(c0, c1, va, vb), xp(c0, c1, va, vb), MAX)  # hi
                me, hi = m1, M1
                # stage H over out cols [w0, w1); reuse dead tiles:
                # pa->xm, pc->xp, q->x0, p->t2 (names = storage). Partials are
                # confined to [w0, w1): the w1'th value is never used, and
                # writing it would clobber the aliased x tiles at the next
                # col-chunk's V halo.
                h1 = w1 + 1
                tt(xm(c0, c1, w0, w1), lo(c0, c1, w0, w1), lo(c0, c1, w0 + 1, h1), MAX)  # pa
                tt(xp(c0, c1, w0, w1), hi(c0, c1, w0, w1), hi(c0, c1, w0 + 1, h1), MIN)  # pc
                tt(x0(c0, c1, w0, w1), me(c0, c1, w0, w1), me(c0, c1, w0 + 1, h1), MAX)  # q
                tt(t2(c0, c1, w0, w1), me(c0, c1, w0, w1), me(c0, c1, w0 + 1, h1), MIN)  # p
                tt(xm(c0, c1, w0, w1), xm(c0, c1, w0, w1), lo(c0, c1, w0 + 2, vb), MAX)  # A
                tt(xp(c0, c1, w0, w1), xp(c0, c1, w0, w1), hi(c0, c1, w0 + 2, vb), MIN)  # Cc
                tt(x0(c0, c1, w0, w1), x0(c0, c1, w0, w1), me(c0, c1, w0 + 2, vb), MIN)  # b1
                tt(x0(c0, c1, w0, w1), t2(c0, c1, w0, w1), x0(c0, c1, w0, w1), MAX)  # B
                A, B, Cc = xm, x0, xp
                tt(lo(c0, c1, w0, w1), A(c0, c1, w0, w1), B(c0, c1, w0, w1), MIN)  # m2
                tt(xm(c0, c1, w0, w1), A(c0, c1, w0, w1), B(c0, c1, w0, w1), MAX)  # M2
                tt(xp(c0, c1, w0, w1), xm(c0, c1, w0, w1), Cc(c0, c1, w0, w1), MIN)  # t3
                if last_piece:
                    # final piece: write f32 directly (fp32 rate on this one
                    # op) into its own tile so earlier stores don't pick up a
                    # false dep on the last DVE op
                    tt(outfB[:, c0:c1, 0 : w1 - w0], xp(c0, c1, w0, w1), lo(c0, c1, w0, w1), MAX)
                else:
                    tt(lo(c0, c1, w0, w1), xp(c0, c1, w0, w1), lo(c0, c1, w0, w1), MAX)  # out

            def emit_compute_super(S, t0):
                """Both bands of super S (bands t0, t0+1) in one 18-op pass."""
                d = sup[S]
                m1, M1, lo, t2 = alloc_scratch()
                tiles = {"xm": d["xm"], "x0": d["x0"], "xp": d["xp"],
                         "m1": m1, "M1": M1, "lo": lo, "t2": t2}
                v = {k: (lambda z: lambda c0, c1, a, b: z[:, :, c0:c1, a:b])(z)
                     for k, z in tiles.items()}
                net(v, None, 0, C, 0, W)
                # steady-state stores ride the SP queue and their f32
                # conversions run on the (otherwise idle) Pool engine: the ACT
                # queue must stay clear for the next bands' input conversions,
                # or the load->convert chain falls behind the 2-band compute
                # period and starves the DVE between supers
                for h, t in ((0, t0), (1, t0 + 1)):
                    emit_stores(
                        t, lo, h,
                        ((0, 2, 0, W, "s", "p"), (2, 3, 0, W, "s", "p")),
                    )

            def emit_compute_half(S, h, t, chunks, store_chunks=None,
                                  fuse_last=False):
                d = sup[S]
                m1, M1, lo, t2 = alloc_scratch()
                tiles = {"xm": d["xm"], "x0": d["x0"], "xp": d["xp"],
                         "m1": m1, "M1": M1, "lo": lo, "t2": t2}
                v = {k: (lambda z: lambda c0, c1, a, b: _hv(z, h, c0, c1, a, b))(z)
                     for k, z in tiles.items()}
                if fuse_last:
                    outfB = opool.tile([P, C, W - TB], F32, tag="outfB", name="outfB")
                else:
                    outfB = None
                for ck in chunks:
                    c0, c1, w0, w1 = ck
                    net(v, None, c0, c1, w0, w1, outfB,
                        last_piece=(fuse_last and ck == chunks[-1]))
                if store_chunks is not None:
                    oc = store_chunks
                elif len(chunks) == 1:
                    oc = ((0, 2, 0, W), (2, 3, 0, W))
                else:
                    oc = chunks
                emit_stores(t, lo, h, oc, outfB, fuse_last)

            full = [(0, C, 0, W)]
            # band 0: small first piece so DVE starts early (load split 2 cols
            # past the compute split so piece 1's V halo stays in load 1)
            load0 = [(0, 1, 0, s0), (0, 1, s0, W), (1, 3, 0, W)]
            comp0 = [(0, 1, 0, s0 - 2), (0, 1, s0 - 2, W), (1, 3, 0, W)]
            # last band: small last piece for a short drain; stores fan out
            # across the ACT, SP and Pool queues so the drains run in parallel
            TB = 928  # tail split col
            tailc = [(0, 3, 0, TB), (0, 3, TB, W)]
            tail_store = [
                (0, 1, 0, TB, "a", "a"),
                (1, 2, 0, TB, "s", "a"),
                (2, 3, 0, TB, "p", "v"),
                (0, 3, TB, W, "a", "a"),
            ]
            # supers: S0 = band 0 (half), S1=(1,2), S2=(3,4), S3=(5,6),
            # S4 = band 7 (half)
            sstore = ((0, 2, 0, W, "s", "p"), (2, 3, 0, W, "s", "p"))
            # band 0's xp loads ride the ACT queue: piece 3's xm/x0 then
            # clear the SP queue sooner, trimming the fill hiccup before its
            # compute (xp is only needed 3 ops into each piece's V stage)
            emit_load(0, 0, 0, load0, xpq=nc.scalar)
            # band 2 rides the Pool queue end-to-end (loads, staging in the
            # idle outf slots, conversions) so it lands in parallel with
            # band 1's SP/ACT chain -- that makes the (1,2) super pass start
            # on time instead of stalling ~10us on band 2's conversions
            emit_load(2, 1, 1, full, lq=nc.gpsimd, cq=nc.gpsimd.tensor_copy)
            emit_load(1, 1, 0, full)
            emit_compute_half(0, 0, 0, comp0)
            emit_load(3, 2, 0, full)
            emit_compute_super(1, 1)
            emit_load(4, 2, 1, full)
            emit_load(5, 3, 0, full)
            emit_compute_super(2, 3)
            emit_load(6, 3, 1, full)
            emit_load(7, 4, 0, full)
            emit_compute_super(3, 5)
            emit_compute_half(4, 0, 7, tailc, tail_store, fuse_last=True)

    _legalize_waits(nc)
    return nc


_NC = None


def kernel(input):
    global _NC
    if _NC is None:
        _NC = build()
    input = np.asarray(input, dtype=np.float32)
    in_maps = [{"x": np.ascontiguousarray(input[i])} for i in range(input.shape[0])]
    res = run_bass_kernel_spmd(_NC, in_maps, core_ids=list(range(len(in_maps))))
    return np.stack([r["y"] for r in res.results], axis=0)
